# revision 10
# baseline (speedup 1.0000x reference)
"""Capsule-routing (ClassCapsLayer) Bass/Tile kernel for 8 trn2 NeuronCores.

Math (reference):
    priors[b,c,r,o] = sum_i x[b,c,r,i] * w[c,r,i,o]
    logits_1 = 0;  logits_{t+1} = logits_t + priors * v_t
    probs_t = softmax_r(logits_t);  s_t = sum_r probs_t * priors
    v_t = squash(s_t)  with GLOBAL Frobenius norm n2 = sum(s_t^2) over (b,c,o)

Key identity: logits_t = priors * W_t with W_t = sum_{u<t} v_u, a per-(b,c,o)
scalar, so each routing iteration is one ACT exp pass (accum_out gives den for
free) + one fused DVE tensor_tensor_reduce (num) per (c,b) tile.

Matmul phase ("quad" packing): each PE stationary tile is [128,128] bf16
holding FOUR routes' weights -- rows 0:63 / 64:127 carry two routes' i-dims
(K halves), cols 0:63 / 64:127 carry two route-pairs' o-dims.  128 weight
columns + bf16 triggers Fast Weight Load (2 elem/cycle), so the full 64 MB
weight stream enters the PE at ~2x the naive rate with 4x fewer instructions.
The moving operand is x zero-interleaved to 32 cols (b-major, 4 route-slots
fastest); output [128, b, 4] per quad has its valid halves written to SBUF
with contiguous runs (b-major routing layout preserved).

Sharding: classes split 4-per-core (weights read exactly once fleet-wide).
The only cross-core quantity is the scalar n2 per iteration -> AllReduce of a
single f32 (warmed up by two early dummy AllReduces so the real ones don't
pay the cold-start / skew cost).  Final squash on host from per-core partial
numerators/denominators.
"""

import numpy as np
import ml_dtypes

import concourse.bass as bass
import concourse.tile as tile
from concourse import bacc, mybir
from concourse.bass import ts
from concourse.bass_utils import run_bass_kernel_spmd

# Full problem dims (hardcoded; kernel.py must be self-contained)
B, C, R, I, O = 8, 32, 2048, 64, 64
NCORES = 8
CL = C // NCORES      # classes per core
RH = R // 2           # route slots per partition-half
NT = 16               # weight tiles per class
QT = 32               # quads per tile (quad = 4 routes)
P = 128

F32 = mybir.dt.float32
BF16 = mybir.dt.bfloat16
AF = mybir.ActivationFunctionType
ALU = mybir.AluOpType
AX = mybir.AxisListType

TRACE = False         # set by test.py to collect HW exec time
TMPDIR = None         # set by test.py to keep NTFF/perfetto artifacts
LAST_RESULT = [None]  # BassKernelResults of the most recent run

_cache = {}


def build(iters, cl=CL, b_dim=B, ncores=NCORES, num_mode="amr", warmup=2,
          den_dve=0):
    nc = bacc.Bacc(
        "TRN2", target_bir_lowering=False, debug=False, num_devices=ncores
    )
    w_in = nc.dram_tensor(
        "w_in", [cl, NT, P, QT, P], BF16, kind="ExternalInput"
    ).ap()
    x_in = nc.dram_tensor(
        "x_in", [cl, NT, P, QT, b_dim, 4], BF16, kind="ExternalInput"
    ).ap()
    f2_in = nc.dram_tensor("f2_in", [P, P], F32, kind="ExternalInput").ap()
    onek_in = nc.dram_tensor("onek_in", [P, 1], F32, kind="ExternalInput").ap()
    onem_in = nc.dram_tensor("onem_in", [1, P], F32, kind="ExternalInput").ap()
    num_o = nc.dram_tensor("num_o", [P, cl, b_dim], F32, kind="ExternalOutput").ap()
    den_o = nc.dram_tensor("den_o", [P, cl, b_dim], F32, kind="ExternalOutput").ap()

    with tile.TileContext(nc) as tc:
        with (
            tc.tile_pool(name="persist", bufs=1) as persist,
            tc.tile_pool(name="wpool", bufs=4) as wpool,
            tc.tile_pool(name="xpool", bufs=3) as xpool,
            tc.tile_pool(name="ppool", bufs=2, space="PSUM") as ppool,
            tc.tile_pool(name="psmall", bufs=1, space="PSUM") as psmall,
            tc.tile_pool(name="scratch", bufs=6) as scratch,
            tc.tile_pool(name="small", bufs=2) as small,
            tc.tile_pool(name="dram", bufs=2, space="DRAM") as dram,
        ):
            # ---- persistent state ----
            # priors in b-major routing layout; slot index = (t, q, s)
            priors = persist.tile([P, cl, b_dim, RH], BF16)
            f2_sb = persist.tile([P, P], F32)
            nc.sync.dma_start(f2_sb[:], f2_in[:])
            onek_sb = persist.tile([P, 1], F32)
            nc.sync.dma_start(onek_sb[:], onek_in[:])
            onem_sb = persist.tile([1, P], F32)
            nc.sync.dma_start(onem_sb[:], onem_in[:])
            w_t = persist.tile([P, cl, b_dim], F32)
            nc.vector.memset(w_t[:], 0.0)
            num_t = persist.tile([P, cl, b_dim], F32)
            numh_t = persist.tile([P, cl, b_dim], F32)
            den_t = persist.tile([P, cl, b_dim], F32)
            nc.vector.memset(den_t[:], float(RH))
            s_t = persist.tile([P, cl, b_dim], F32)
            sq_t = persist.tile([P, cl, b_dim], F32)
            sacc4 = persist.tile([P, cl], F32)
            zsb = persist.tile([1, 1], F32)
            nc.vector.memset(zsb[:], 0.0)
            zback = persist.tile([1, 2], F32)

            # ---- CC warm-up: absorb the cold-start cost of the collective
            # path (and sync the cores) while the matmul phase runs ----
            for d in range(warmup):
                wci = dram.tile([1, 1], F32, tag="ccin")
                wco = dram.tile([1, 1], F32, tag="ccout")
                nc.sync.dma_start(wci[:], zsb[:])
                nc.gpsimd.collective_compute(
                    "AllReduce", ALU.add,
                    replica_groups=[list(range(ncores))],
                    ins=[wci.opt()], outs=[wco.opt()],
                )
                nc.sync.dma_start(zback[:, d : d + 1], wco[:])

            # ---- priors matmul phase ----
            # One [128,128] stationary tile = 4 routes (FWL-eligible); moving
            # x is zero-interleaved to 32 cols; PSUM out [128, b, 4] per quad.
            for c in range(cl):
                for t in range(NT):
                    wb = wpool.tile([P, QT, P], BF16, tag="wb")
                    nc.sync.dma_start(wb[:], w_in[c, t])
                    xs = xpool.tile([P, QT, b_dim, 4], BF16, tag="xs")
                    nc.sync.dma_start(xs[:], x_in[c, t])
                    pt = ppool.tile([P, QT, b_dim, 4], F32, tag="pt")
                    for q in range(QT):
                        nc.tensor.matmul(
                            pt[:, q], wb[:, q], xs[:, q], start=True, stop=True
                        )
                    # valid halves -> SBUF (b-major, contiguous 128B runs)
                    dst0 = priors[0:64, c, :, ts(t, 64)].rearrange(
                        "p b (q s) -> p b q s", s=2
                    )
                    dst1 = priors[64:128, c, :, ts(t, 64)].rearrange(
                        "p b (q s) -> p b q s", s=2
                    )
                    nc.vector.tensor_copy(
                        dst0, pt[0:64, :, :, 0:2].transpose([0, 2, 1, 3])
                    )
                    nc.scalar.copy(
                        dst1, pt[64:128, :, :, 2:4].transpose([0, 2, 1, 3])
                    )
                    # iteration-1 numerator partials: first half of the
                    # class's slots reduce while its second half still streams
                    if t == NT // 2 - 1:
                        for b in range(b_dim):
                            nc.vector.tensor_reduce(
                                numh_t[:, c, b : b + 1],
                                priors[:, c, b, 0 : RH // 2],
                                AX.X,
                                ALU.add,
                            )
                # second-half reduce + combine (overlaps next class's matmuls)
                for b in range(b_dim):
                    nc.vector.tensor_reduce(
                        num_t[:, c, b : b + 1],
                        priors[:, c, b, RH // 2 : RH],
                        AX.X,
                        ALU.add,
                    )
                nc.vector.tensor_add(
                    num_t[:, c], num_t[:, c], numh_t[:, c]
                )
                # per-class fold + squared partial (s1 = fold(num)/R)
                nf_c = psmall.tile([P, cl, b_dim], F32, tag="nf")
                nc.tensor.matmul(
                    nf_c[:, c], f2_sb[:], num_t[:, c], start=True, stop=True
                )
                nc.vector.tensor_scalar_mul(s_t[:, c], nf_c[:, c], 1.0 / R)
                nc.scalar.activation(
                    sq_t[:, c], s_t[:, c], AF.Square,
                    accum_out=sacc4[:, c : c + 1],
                )

            # ---- routing iterations ----
            for it in range(iters):
                if it > 0:
                    # e = exp(priors * W_t); den via ACT accumulate,
                    # num via fused multiply-reduce on DVE
                    kcb = 0
                    for c in range(cl):
                        for b in range(b_dim):
                            # balance den between ACT accumulate (free but
                            # serializes a read-accumulator op on Scalar) and
                            # a DVE reduce, so neither engine is the bottleneck
                            on_dve = (kcb * den_dve) % 32 >= 32 - den_dve
                            kcb += 1
                            e_t = scratch.tile([P, RH], BF16, tag="e")
                            nc.scalar.activation(
                                e_t[:],
                                priors[:, c, b],
                                AF.Exp,
                                scale=w_t[:, c, b : b + 1],
                                accum_out=(
                                    None if on_dve
                                    else den_t[:, c, b : b + 1]
                                ),
                            )
                            if on_dve:
                                # den = sum(e) as (p*0+1)*e fused on DVE
                                d_t = scratch.tile([P, RH], BF16, tag="dt")
                                nc.vector.affine_mul_reduce(
                                    out=d_t[:],
                                    accum_out=den_t[:, c, b : b + 1],
                                    in0=priors[:, c, b],
                                    in1=e_t[:],
                                    scale=0.0,
                                    bias=1.0,
                                )
                            if num_mode == "amr":
                                # num = sum(e * priors) in one fused DVE op
                                t_t = scratch.tile([P, RH], BF16, tag="tt")
                                nc.vector.affine_mul_reduce(
                                    out=t_t[:],
                                    accum_out=num_t[:, c, b : b + 1],
                                    in0=e_t[:],
                                    in1=priors[:, c, b],
                                    scale=1.0,
                                    bias=0.0,
                                )
                            elif num_mode == "dvemul":
                                t_t = scratch.tile([P, RH], BF16, tag="tt")
                                nc.vector.tensor_mul(
                                    t_t[:], e_t[:], priors[:, c, b]
                                )
                                nc.vector.tensor_reduce(
                                    num_t[:, c, b : b + 1], t_t[:],
                                    AX.X, ALU.add,
                                )
                            else:
                                t_t = scratch.tile([P, RH], BF16, tag="tt")
                                nc.gpsimd.tensor_mul(
                                    t_t[:], e_t[:], priors[:, c, b]
                                )
                                nc.vector.tensor_reduce(
                                    num_t[:, c, b : b + 1], t_t[:],
                                    AX.X, ALU.add,
                                )

                if it == iters - 1:
                    nc.sync.dma_start(num_o[:], num_t[:])
                    nc.sync.dma_start(den_o[:], den_t[:])
                    continue

                # ---- fold halves + squash + AllReduce of n2 ----
                # F2[k,m] = (k%64 == m%64) sums the two route-halves and
                # duplicates the result into both halves.
                if it > 0:
                    nf = psmall.tile([P, cl, b_dim], F32, tag="nf")
                    df = psmall.tile([P, cl, b_dim], F32, tag="df")
                    nc.tensor.matmul(
                        nf[:], f2_sb[:], num_t[:], start=True, stop=True
                    )
                    nc.tensor.matmul(
                        df[:], f2_sb[:], den_t[:], start=True, stop=True
                    )
                    rd_t = small.tile([P, cl, b_dim], F32, tag="rd")
                    nc.vector.reciprocal(rd_t[:], df[:])
                    nc.vector.tensor_mul(s_t[:], nf[:], rd_t[:])
                    # n2_partial = sum(s^2)/2 (each value in both halves)
                    sacc = small.tile([P, 1], F32, tag="sacc")
                    nc.scalar.activation(
                        sq_t[:], s_t[:], AF.Square, accum_out=sacc[:]
                    )
                else:
                    # partials were accumulated per class during the matmul
                    # phase; just sum the 4 class columns
                    sacc = small.tile([P, 1], F32, tag="sacc")
                    nc.vector.tensor_reduce(sacc[:], sacc4[:], AX.X, ALU.add)
                n2p = psmall.tile([1, 1], F32, tag="n2p")
                nc.tensor.matmul(n2p[:], onek_sb[:], sacc[:], start=True, stop=True)
                n2sb = small.tile([1, 1], F32, tag="n2sb")
                nc.vector.tensor_copy(n2sb[:], n2p[:])
                cc_in = dram.tile([1, 1], F32, tag="ccin")
                cc_out = dram.tile([1, 1], F32, tag="ccout")
                nc.sync.dma_start(cc_in[:], n2sb[:])
                nc.gpsimd.collective_compute(
                    "AllReduce",
                    ALU.add,
                    replica_groups=[list(range(ncores))],
                    ins=[cc_in.opt()],
                    outs=[cc_out.opt()],
                )
                n2g = small.tile([1, 1], F32, tag="n2g")
                nc.sync.dma_start(n2g[:], cc_out[:])

                # squash scale g = sqrt(n2)/(1+n2), n2 = 0.5*allreduced.
                # sqrt via exp(0.5*ln(.)) keeps ACT on the ln/exp table set.
                l_t = small.tile([1, 1], F32, tag="lt")
                nc.scalar.activation(l_t[:], n2g[:], AF.Ln, scale=0.5)
                r_t = small.tile([1, 1], F32, tag="rt")
                nc.scalar.activation(r_t[:], l_t[:], AF.Exp, scale=0.5)
                t1_t = small.tile([1, 1], F32, tag="t1")
                nc.vector.tensor_scalar(
                    t1_t[:], n2g[:], 0.5, 1.0, ALU.mult, ALU.add
                )
                u_t = small.tile([1, 1], F32, tag="ut")
                nc.vector.reciprocal(u_t[:], t1_t[:])
                g_t = small.tile([1, 1], F32, tag="g")
                nc.vector.tensor_mul(g_t[:], r_t[:], u_t[:])
                # broadcast g to all partitions via K=1 matmul with ones
                gb_ps = psmall.tile([P, 1], F32, tag="gb")
                nc.tensor.matmul(gb_ps[:], onem_sb[:], g_t[:], start=True, stop=True)
                gb_sb = small.tile([P, 1], F32, tag="gbs")
                nc.vector.tensor_copy(gb_sb[:], gb_ps[:])
                # v = g*s ; W += v
                if it == 0:
                    nc.vector.tensor_scalar_mul(w_t[:], s_t[:], gb_sb[:])
                else:
                    v_t = small.tile([P, cl, b_dim], F32, tag="v")
                    nc.vector.tensor_scalar_mul(v_t[:], s_t[:], gb_sb[:])
                    nc.vector.tensor_add(w_t[:], w_t[:], v_t[:])

    nc.compile()
    return nc


def prep_inputs(x, w, cl=CL, b_dim=B, ncores=NCORES):
    """Host-side relayout into quad-packed tiles (f32 -> bf16)."""
    ctot = cl * ncores
    # w: [C, R, I, O]; route r = 4*(32t+q) + 2p + a
    # W_in[c, t, 64a+i, q, 64p+o] = w[c, r, i, o]
    w6 = w.reshape(ctot, NT, QT, 2, 2, 64, 64)         # (c,t,q,p,a,i,o)
    wb = np.ascontiguousarray(
        w6.transpose(0, 1, 4, 5, 2, 3, 6)              # (c,t,a,i,q,p,o)
    ).reshape(ctot, NT, P, QT, P).astype(ml_dtypes.bfloat16)
    # x: [B, C, R, 1, I]; X_in[c, t, 64a+i, q, b, 2p+a2] nonzero iff a2 == a
    x6 = x[:, :, :, 0, :].reshape(b_dim, ctot, NT, QT, 2, 2, 64)  # (b,c,t,q,p,a,i)
    xp = np.zeros((ctot, NT, 2, 64, QT, b_dim, 2, 2), np.float32)
    for a in range(2):
        for p in range(2):
            xp[:, :, a, :, :, :, p, a] = x6[:, :, :, :, p, a, :].transpose(
                1, 2, 4, 3, 0
            )
    xb = xp.reshape(ctot, NT, P, QT, b_dim, 4).astype(ml_dtypes.bfloat16)
    f2 = np.equal.outer(np.arange(P) % 64, np.arange(P) % 64).astype(np.float32)
    onek = np.ones((P, 1), np.float32)
    onem = np.ones((1, P), np.float32)
    in_maps = []
    for k in range(ncores):
        in_maps.append(
            {
                "w_in": np.ascontiguousarray(wb[k * cl : (k + 1) * cl]),
                "x_in": np.ascontiguousarray(xb[k * cl : (k + 1) * cl]),
                "f2_in": f2,
                "onek_in": onek,
                "onem_in": onem,
            }
        )
    return in_maps


def postprocess(results, cl=CL, b_dim=B, ncores=NCORES):
    """Fold halves, divide, global squash -> v [B, C, 1, 1, O] f32."""
    ctot = cl * ncores
    s = np.empty((b_dim, ctot, 64), np.float32)
    for k in range(ncores):
        num = np.asarray(results[k]["num_o"], np.float32)  # [P, cl, B]
        den = np.asarray(results[k]["den_o"], np.float32)
        sk = (num[:64] + num[64:]) / (den[:64] + den[64:])  # [64(o), cl, B]
        s[:, k * cl : (k + 1) * cl, :] = sk.transpose(2, 1, 0)
    n2 = np.sum(s.astype(np.float32) ** 2, dtype=np.float32)
    g = np.float32(np.sqrt(n2) / (1.0 + n2))
    v = (g * s).astype(np.float32)
    return v[:, :, None, None, :]


def kernel(x, route_weights, iterations):
    iters = int(iterations)
    assert iters >= 1
    x = np.asarray(x, dtype=np.float32)
    w = np.asarray(route_weights, dtype=np.float32)
    if iters not in _cache:
        _cache[iters] = build(iters)
    nc = _cache[iters]
    in_maps = prep_inputs(x, w)
    res = run_bass_kernel_spmd(
        nc, in_maps, list(range(NCORES)), trace=TRACE, tmpdir=TMPDIR
    )
    LAST_RESULT[0] = res
    return postprocess(res.results)
